# revision 1
# baseline (speedup 1.0000x reference)
"""VQ codebook EMA kernel for 8 Trainium2 NeuronCores.

Data-parallel: x [64,256,32,32] sharded over batch (8 b-blocks/core);
codebook [256,1024] replicated; per-core cluster counts + centroid sums
all-reduced on device before the EMA normalize and gather.
"""
import sys
sys.path.insert(0, "/opt/pypackages")
sys.path.insert(0, "/opt/trn_rl_repo")
import os
import numpy as np
import concourse.bass as bass
import concourse.mybir as mybir
import concourse.tile as tile
from concourse import bacc, bass_isa
from concourse.bass_utils import run_bass_kernel_spmd
from concourse.masks import make_identity

N_CORES = 8
B, C, H, W = 64, 256, 32, 32
F, K = 256, 1024
B_LOC = B // N_CORES           # 8 b-blocks per core
HW = H * W                     # 1024 tokens per b-block
N_CHUNK = B_LOC * (HW // 128)  # 64 chunks of 128 tokens
N_TOK = N_CHUNK * 128          # 8192 tokens per core
BIG = 16384.0                  # 2^14: exact scaling; +1 survives ulp(BIG*m)
DECAY = 0.99
EPS = 1e-05

f32 = mybir.dt.float32
f16 = mybir.dt.float16
i16 = mybir.dt.int16
u32 = mybir.dt.uint32

KSTAGE = int(os.environ.get("KSTAGE", "5"))
_NC = None


def _build():
    nc = bacc.Bacc("TRN2", target_bir_lowering=False, debug=False,
                   num_devices=N_CORES)
    x_d = nc.dram_tensor("x", [B_LOC, C, H, W], f32, kind="ExternalInput").ap()
    cent_d = nc.dram_tensor("centroids", [C, K], f32, kind="ExternalInput").ap()
    cs_d = nc.dram_tensor("cluster_size", [K], f32, kind="ExternalInput").ap()
    avg_d = nc.dram_tensor("centroids_avg", [C, K], f32, kind="ExternalInput").ap()
    out_d = nc.dram_tensor("out", [B_LOC, C, H, W], f32, kind="ExternalOutput").ap()

    x_v = x_d.rearrange("b (i p) h w -> b i p (h w)", p=128)     # [8, 2, 128, 1024]
    cent_v = cent_d.rearrange("(i p) k -> i p k", p=128)          # [2, 128, 1024]
    avg_v = avg_d.rearrange("(i p) k -> i p k", p=128)
    cs8_v = cs_d.rearrange("(s p) -> s p", p=128)                 # [8, 128]
    out_v = out_d.rearrange("b (i p) h w -> b i p (h w)", p=128)

    with tile.TileContext(nc, num_cores=N_CORES) as tc:
        with (
            tc.tile_pool(name="const", bufs=1) as constp,
            tc.tile_pool(name="xpool", bufs=2) as xpool,
            tc.tile_pool(name="work", bufs=1) as work,
            tc.tile_pool(name="small", bufs=2) as small,
            tc.tile_pool(name="dram", bufs=1, space="DRAM") as dram,
        ):
            # ---------------- constants / setup ----------------
            ident = constp.tile([128, 128], f32)
            make_identity(nc, ident[:])
            ones_row = constp.tile([1, 128], f32)
            nc.vector.memset(ones_row[:], 1.0)
            ones_col32 = constp.tile([128, 1], f32)
            nc.vector.memset(ones_col32[:], 1.0)
            ones_col16 = constp.tile([128, 1], f16)
            nc.vector.memset(ones_col16[:], 1.0)

            cents = [constp.tile([128, K], f32, name=f"cent{i}") for i in range(2)]
            cents2 = [constp.tile([128, K], f32, name=f"cent2{i}") for i in range(2)]
            for i in range(2):
                nc.sync.dma_start(cents[i][:], cent_v[i])
                nc.vector.tensor_scalar_mul(cents2[i][:], cents[i][:], 2.0)

            cs8 = constp.tile([8, 128], f32)       # cluster_size as [s, p]
            nc.sync.dma_start(cs8[:], cs8_v)
            avgs = [constp.tile([128, K], f32, name=f"avg{i}") for i in range(2)]
            for i in range(2):
                nc.sync.dma_start(avgs[i][:], avg_v[i])

            ind_all = constp.tile([128, N_CHUNK], u32)

            ccin = dram.tile([257, K], f32)
            ccout = dram.tile([257, K], f32, addr_space="Shared")
            tab16_dram = dram.tile([K, F], f16)

            with tc.tile_pool(name="psum1", bufs=1, space="PSUM") as psum1:
                # ||c||^2 -> negc2 row (uses the S slot before the loop)
                c2ps = psum1.tile([1, K], f32, tag="S", name="c2ps")
                sq = work.tile([128, K], f32, tag="sq")
                for i in range(2):
                    nc.vector.tensor_tensor(out=sq[:], in0=cents[i][:],
                                            in1=cents[i][:],
                                            op=mybir.AluOpType.mult)
                    for h in range(2):
                        nc.tensor.matmul(c2ps[:, h*512:(h+1)*512], ones_col32[:],
                                         sq[:, h*512:(h+1)*512],
                                         start=(i == 0), stop=(i == 1))
                negc2 = constp.tile([1, K], f32)
                nc.vector.tensor_scalar_mul(negc2[:], c2ps[:], -1.0)

                segps = [psum1.tile([128, K], f32, name=f"segp{i}") for i in range(2)]
                cntps = psum1.tile([1, K], f32, tag="cntps", name="cntps")

                # ---------------- phase 1: 64 chunks ----------------
                for bi in range(B_LOC):
                    xts = [xpool.tile([128, HW], f32, name=f"xt{i}", tag=f"xt{i}")
                           for i in range(2)]
                    xt16s = [xpool.tile([128, HW], f16, name=f"xt16{i}",
                                        tag=f"xt16{i}") for i in range(2)]
                    xf16 = xpool.tile([128, 8 * F], f16, tag="xf16")
                    for i in range(2):
                        nc.sync.dma_start(xts[i][:], x_v[bi, i])
                        nc.scalar.copy(xt16s[i][:], xts[i][:])
                    for t in range(8):
                        for i in range(2):
                            nc.sync.dma_start_transpose(
                                xf16[:, t*F + i*128: t*F + (i+1)*128],
                                xt16s[i][:, t*128:(t+1)*128])

                    for t in range(8):
                        ci = bi * 8 + t
                        S = psum1.tile([128, K], f32, tag="S", name=f"S_{ci}")
                        for h in range(2):
                            hs = slice(h*512, (h+1)*512)
                            for i in range(2):
                                nc.tensor.matmul(S[:, hs],
                                                 xts[i][:, t*128:(t+1)*128],
                                                 cents2[i][:, hs],
                                                 start=(i == 0), stop=False)
                            nc.tensor.matmul(S[:, hs], ones_row[:], negc2[:, hs],
                                             start=False, stop=True)

                        m8 = small.tile([128, 8], f32, tag="m8")
                        nc.vector.max(out=m8[:], in_=S[:])
                        bias = small.tile([128, 1], f32, tag="bias")
                        nc.vector.tensor_scalar(out=bias[:], in0=m8[:, 0:1],
                                                scalar1=-BIG, scalar2=1.0,
                                                op0=mybir.AluOpType.mult,
                                                op1=mybir.AluOpType.add)
                        onehot = work.tile([128, K], f16, tag="onehot", bufs=3)
                        nc.scalar.activation(onehot[:], S[:],
                                             mybir.ActivationFunctionType.Relu,
                                             bias=bias[:], scale=BIG)
                        i8 = small.tile([128, 8], u32, tag="i8")
                        nc.vector.max_index(out=i8[:], in_max=m8[:], in_values=S[:])
                        nc.vector.tensor_copy(ind_all[:, ci:ci+1], i8[:, 0:1])

                        for i in range(2):
                            for h in range(2):
                                nc.tensor.matmul(
                                    segps[i][:, h*512:(h+1)*512],
                                    xf16[:, t*F + i*128: t*F + (i+1)*128],
                                    onehot[:, h*512:(h+1)*512],
                                    start=(ci == 0), stop=(ci == N_CHUNK - 1),
                                    skip_group_check=True)
                        for h in range(2):
                            nc.tensor.matmul(cntps[:, h*512:(h+1)*512],
                                             ones_col16[:],
                                             onehot[:, h*512:(h+1)*512],
                                             start=(ci == 0),
                                             stop=(ci == N_CHUNK - 1),
                                             skip_group_check=True)

                # ------- flush partials (scaled by 1-decay) -------
                for i in range(2):
                    fl = work.tile([128, K], f32, name=f"fl{i}", tag="flush",
                                   bufs=2)
                    nc.vector.tensor_scalar_mul(fl[:], segps[i][:], 1.0 - DECAY)
                    nc.sync.dma_start(ccin[i*128:(i+1)*128, :], fl[:])
                cfl = work.tile([1, K], f32, tag="cflush")
                nc.vector.tensor_scalar_mul(cfl[:], cntps[:], 1.0 - DECAY)
                nc.sync.dma_start(ccin[256:257, :], cfl[:])

            # psum1 released; allreduce overlaps the wrapped-idx build
            if KSTAGE >= 2:
                nc.gpsimd.collective_compute(
                    "AllReduce", mybir.AluOpType.add,
                    replica_groups=[list(range(N_CORES))],
                    ins=[ccin.opt()], outs=[ccout.opt()],
                )
            else:
                nc.sync.dma_start(ccout[:], ccin[:])

            # ---- wrapped gather indices (independent of the collective) ----
            # gather slot (r, j) with j = u*64 + c holds ind of token
            # t = 128*c + 16*u + r  -> 8 contiguous [16, 64] copies.
            ind16 = constp.tile([128, N_CHUNK], i16)
            nc.vector.tensor_copy(ind16[:], ind_all[:])
            wrapped = constp.tile([128, N_TOK // 16], i16)
            for u in range(8):
                nc.sync.dma_start(wrapped[0:16, 64*u:64*(u+1)],
                                  ind16[16*u:16*(u+1), :])
            for g in range(1, 8):
                nc.sync.dma_start(wrapped[g*16:(g+1)*16, :], wrapped[0:16, :])

            with tc.tile_pool(name="psum2", bufs=2, space="PSUM") as psum2:
                # ---- EMA + normalize ----
                seg_g = [work.tile([128, K], f32, name=f"segg{i}", tag=f"segg{i}")
                         for i in range(2)]
                for i in range(2):
                    nc.sync.dma_start(seg_g[i][:], ccout[i*128:(i+1)*128, :])
                cnt8 = small.tile([8, 128], f32, tag="cnt8")
                nc.sync.dma_start(cnt8[:], ccout[256:257, :].rearrange(
                    "one (s p) -> (one s) p", p=128))
                cntT_ps = psum2.tile([128, 8], f32, tag="cntT_ps", bufs=1)
                nc.tensor.transpose(cntT_ps[:], cnt8[:], ident[0:8, 0:8])
                cntT = small.tile([128, 8], f32, tag="cntT")
                nc.vector.tensor_copy(cntT[:], cntT_ps[:])
                cs8T_ps = psum2.tile([128, 8], f32, tag="cs8T_ps", bufs=1)
                nc.tensor.transpose(cs8T_ps[:], cs8[:], ident[0:8, 0:8])

                new_csT = small.tile([128, 8], f32, tag="new_csT")
                nc.vector.tensor_scalar_mul(new_csT[:], cs8T_ps[:], DECAY)
                nc.vector.tensor_add(new_csT[:], new_csT[:], cntT[:])
                psum_n = small.tile([128, 1], f32, tag="psum_n")
                nc.vector.reduce_sum(psum_n[:], new_csT[:],
                                     axis=mybir.AxisListType.X)
                n_all = small.tile([128, 1], f32, tag="n_all")
                if KSTAGE >= 3:
                    nc.gpsimd.partition_all_reduce(n_all[:], psum_n[:], channels=128,
                                                   reduce_op=bass_isa.ReduceOp.add)
                else:
                    nc.vector.memset(n_all[:], 550.0)
                # M[k] = 1/cs_norm[k] = (n + K*eps)/n * 1/(new_cs + eps)
                denom = small.tile([128, 1], f32, tag="denom")
                nc.vector.tensor_scalar_add(denom[:], n_all[:], float(K) * EPS)
                rcp_n = small.tile([128, 1], f32, tag="rcp_n")
                nc.vector.reciprocal(rcp_n[:], n_all[:])
                fmul = small.tile([128, 1], f32, tag="fmul")
                nc.vector.tensor_mul(fmul[:], denom[:], rcp_n[:])
                t1 = small.tile([128, 8], f32, tag="t1")
                nc.vector.tensor_scalar_add(t1[:], new_csT[:], EPS)
                r1 = small.tile([128, 8], f32, tag="r1")
                nc.vector.reciprocal(r1[:], t1[:])
                Mt = small.tile([128, 8], f32, tag="Mt")
                nc.vector.tensor_scalar_mul(Mt[:], r1[:], fmul[:])

                newavg = [work.tile([128, K], f32, name=f"newavg{i}",
                                    tag=f"nav{i}") for i in range(2)]
                for i in range(2):
                    nc.vector.tensor_scalar_mul(newavg[i][:], avgs[i][:], DECAY)
                    nc.vector.tensor_add(newavg[i][:], newavg[i][:], seg_g[i][:])

                # ---- table: new_centroids^T [K, F] fp16 in DRAM ----
                tabv = tab16_dram.rearrange("(s p) f -> p s f", p=128)
                for s in range(8):
                    tab_sb = work.tile([128, F], f16, tag="tab_sb", bufs=2)
                    for hh in range(2):
                        tps = psum2.tile([128, 128], f32, tag="tps",
                                         name=f"tps{s}_{hh}")
                        nc.tensor.transpose(tps[:],
                                            newavg[hh][:, s*128:(s+1)*128],
                                            ident[:])
                        nc.vector.tensor_scalar_mul(tab_sb[:, hh*128:(hh+1)*128],
                                                    tps[:], Mt[:, s:s+1])
                    nc.sync.dma_start(tabv[:, s, :], tab_sb[:])

                # ---- phase 2: one gather + permuted convert + output ----
                # 16 gather calls of 512 idxs (>=1024 per call overruns the
                # SWDGE descriptor fifo). call q=(u,ch); i_loc=(bl cc r).
                gath = work.tile([128, 16, 2, 512], f16, tag="gath")
                wv = wrapped.rearrange("p (q j) -> p q j", q=16)
                for q in range(16):
                    nc.gpsimd.dma_gather(gath[:, q], tab16_dram, wv[:, q],
                                         num_idxs=512, num_idxs_reg=512,
                                         elem_size=F, transpose=True)
                gv = gath.rearrange("p (u ch) g (bl cc r) -> p g ch bl cc u r",
                                    u=8, ch=2, bl=4, cc=8, r=16)
                for bi in range(B_LOC):
                    for i in range(2):
                        conv = work.tile([128, 8, 8, 16], f32, name=f"conv{i}",
                                         tag=f"conv{i}", bufs=2)
                        if i == 0:
                            nc.vector.tensor_copy(conv[:], gv[:, i, bi // 4, bi % 4])
                        else:
                            nc.scalar.copy(conv[:], gv[:, i, bi // 4, bi % 4])
                        nc.sync.dma_start(out_v[bi, i],
                                          conv.rearrange("p a b c -> p (a b c)"))

    nc.finalize()
    return nc


def _get_nc():
    global _NC
    if _NC is None:
        _NC = _build()
    return _NC


def kernel(x, centroids, cluster_size, centroids_avg):
    x = np.ascontiguousarray(np.asarray(x, dtype=np.float32))
    centroids = np.ascontiguousarray(np.asarray(centroids, dtype=np.float32))
    cluster_size = np.ascontiguousarray(np.asarray(cluster_size, dtype=np.float32))
    centroids_avg = np.ascontiguousarray(np.asarray(centroids_avg, dtype=np.float32))
    nc = _get_nc()
    in_maps = []
    for i in range(N_CORES):
        in_maps.append({
            "x": x[i*B_LOC:(i+1)*B_LOC],
            "centroids": centroids,
            "cluster_size": cluster_size,
            "centroids_avg": centroids_avg,
        })
    res = run_bass_kernel_spmd(nc, in_maps, core_ids=list(range(N_CORES)))
    out = np.concatenate([res.results[i]["out"] for i in range(N_CORES)], axis=0)
    return out


if __name__ == "__main__":
    rng = np.random.default_rng(0)
    xs = rng.standard_normal((B, C, H, W), dtype=np.float32)
    cs = rng.standard_normal((C, K), dtype=np.float32)
    sz = rng.random(K, dtype=np.float32)
    av = rng.standard_normal((C, K), dtype=np.float32)
    out = kernel(xs, cs, sz, av)
    print("out", out.shape, out.dtype)



# revision 10
# speedup vs baseline: 1.9166x; 1.9166x over previous
"""VQ codebook EMA kernel for 8 Trainium2 NeuronCores.

Data-parallel: x [64,256,32,32] sharded over batch (8 b-blocks/core);
codebook [256,1024] replicated; per-core cluster counts + centroid sums
all-reduced (fp16) on device before the EMA normalize and gather.

v2: distance matmuls in float32r (1 cyc/row vs fp32's 4), -||c||^2 added
on DVE (frees PSUM S tile early, keeps PE 100% busy), counts kept as f16
matmuls, fp16 allreduce payload, transpose=False SWDGE gather (1 desc/idx)
with PE transposes for the output layout.
"""
import sys
sys.path.insert(0, "/opt/pypackages")
sys.path.insert(0, "/opt/trn_rl_repo")
import os
import numpy as np
import concourse.bass as bass
import concourse.mybir as mybir
import concourse.tile as tile
from concourse import bacc, bass_isa
from concourse.bass_utils import run_bass_kernel_spmd
from concourse.masks import make_identity

N_CORES = 8
B, C, H, W = 64, 256, 32, 32
F, K = 256, 1024
B_LOC = B // N_CORES           # 8 b-blocks per core
HW = H * W                     # 1024 tokens per b-block
N_CHUNK = B_LOC * (HW // 128)  # 64 chunks of 128 tokens
N_TOK = N_CHUNK * 128          # 8192 tokens per core
BIG = 16384.0                  # 2^14: exact scaling; +1 survives ulp(BIG*m)
DECAY = 0.99
EPS = 1e-05

f32 = mybir.dt.float32
f32r = mybir.dt.float32r
f16 = mybir.dt.float16
i16 = mybir.dt.int16
u16 = mybir.dt.uint16

KSTAGE = int(os.environ.get("KSTAGE", "5"))
DIST_F32 = int(os.environ.get("DIST_F32", "0"))     # 1 -> exact fp32 distances
GQ = int(os.environ.get("GQ", "1024"))              # gather idxs per SWDGE call
_NC = None


def _build():
    nc = bacc.Bacc("TRN2", target_bir_lowering=False, debug=False,
                   num_devices=N_CORES)
    x_d = nc.dram_tensor("x", [B_LOC, C, H, W], f32, kind="ExternalInput").ap()
    cent_d = nc.dram_tensor("centroids", [C, K], f32, kind="ExternalInput").ap()
    cs_d = nc.dram_tensor("cluster_size", [K], f32, kind="ExternalInput").ap()
    avg_d = nc.dram_tensor("centroids_avg", [C, K], f32, kind="ExternalInput").ap()
    out_d = nc.dram_tensor("out", [B_LOC, C, H, W], f32, kind="ExternalOutput").ap()

    x_v = x_d.rearrange("b (i p) h w -> b i p (h w)", p=128)     # [8, 2, 128, 1024]
    cent_v = cent_d.rearrange("(i p) k -> i p k", p=128)          # [2, 128, 1024]
    avg_v = avg_d.rearrange("(i p) k -> i p k", p=128)
    cs8_v = cs_d.rearrange("(s p) -> s p", p=128)                 # [8, 128]
    out_v = out_d.rearrange("b (i p) h w -> b i p (h w)", p=128)

    mmdt = f32 if DIST_F32 else f32r

    with tile.TileContext(nc, num_cores=N_CORES) as tc:
        with (
            tc.tile_pool(name="const", bufs=1) as constp,
            tc.tile_pool(name="xpool", bufs=2) as xpool,
            tc.tile_pool(name="work", bufs=1) as work,
            tc.tile_pool(name="small", bufs=2) as small,
            tc.tile_pool(name="dram", bufs=1, space="DRAM") as dram,
        ):
            # ---------------- constants / setup ----------------
            ident = constp.tile([128, 128], f32)
            make_identity(nc, ident[:])
            ident16 = constp.tile([128, 128], f16)
            nc.scalar.copy(ident16[:], ident[:])
            ones_row = constp.tile([1, 128], f32)
            nc.vector.memset(ones_row[:], 1.0)
            ones_col32 = constp.tile([128, 1], f32)
            nc.vector.memset(ones_col32[:], 1.0)
            ones_col16 = constp.tile([128, 1], f16)
            nc.vector.memset(ones_col16[:], 1.0)

            cents2 = [constp.tile([128, K], f32, name=f"cent2{i}") for i in range(2)]
            for i in range(2):
                # load c then scale to 2c in place
                nc.sync.dma_start(cents2[i][:], cent_v[i])

            cs8 = constp.tile([8, 128], f32)       # cluster_size as [s, p]
            nc.sync.dma_start(cs8[:], cs8_v)
            avgs = [constp.tile([128, K], f32, name=f"avg{i}") for i in range(2)]
            for i in range(2):
                nc.sync.dma_start(avgs[i][:], avg_v[i])

            ind_all = constp.tile([128, N_CHUNK], u16)
            wrapped16 = constp.tile([128, N_TOK // 16], i16)
            negc2bc = constp.tile([128, K], f32)

            ccin = dram.tile([257, K], f16)
            ccout = dram.tile([257, K], f16, addr_space="Shared")
            tab16_dram = dram.tile([K, F], f16)

            with tc.tile_pool(name="psum1", bufs=1, space="PSUM") as psum1:
                # ||c||^2 -> negc2bc [128, K] f32 (uses seg0's banks pre-loop)
                c2ps = psum1.tile([1, K], f32, tag="seg0", name="c2ps")
                sq = work.tile([128, K], f32, tag="sq")
                for i in range(2):
                    nc.vector.tensor_tensor(out=sq[:], in0=cents2[i][:],
                                            in1=cents2[i][:],
                                            op=mybir.AluOpType.mult)
                    for h in range(2):
                        nc.tensor.matmul(c2ps[:, h*512:(h+1)*512], ones_col32[:],
                                         sq[:, h*512:(h+1)*512],
                                         start=(i == 0), stop=(i == 1))
                negc2 = constp.tile([1, K], f32)
                nc.vector.tensor_scalar_mul(negc2[:], c2ps[:], -1.0)
                bc_ps = psum1.tile([128, K], f32, tag="seg1", name="bc_ps")
                for h in range(2):
                    nc.tensor.matmul(bc_ps[:, h*512:(h+1)*512], ones_row[:],
                                     negc2[:, h*512:(h+1)*512],
                                     start=True, stop=True)
                nc.vector.tensor_copy(negc2bc[:], bc_ps[:])
                # 2c rounded to f32r for the distance matmuls
                cents2r = [constp.tile([128, K], mmdt, name=f"cent2r{i}")
                           for i in range(2)]
                for i in range(2):
                    nc.vector.tensor_scalar_mul(cents2r[i][:], cents2[i][:], 2.0)

                segps = [psum1.tile([128, K], f32, tag=f"seg{i}", name=f"segp{i}")
                         for i in range(2)]
                cntps = psum1.tile([1, K], f32, tag="cnt", name="cntps")

                # ---------------- phase 1: 64 chunks ----------------
                for bi in range(B_LOC):
                    xts = [xpool.tile([128, HW], f32, name=f"xt{i}", tag=f"xt{i}")
                           for i in range(2)]
                    xt16s = [xpool.tile([128, HW], f16, name=f"xt16{i}",
                                        tag=f"xt16{i}") for i in range(2)]
                    xf16 = xpool.tile([128, 8 * F], f16, tag="xf16")
                    xrs = [xpool.tile([128, HW], mmdt, name=f"xr{i}",
                                      tag=f"xr{i}") for i in range(2)]
                    for i in range(2):
                        nc.sync.dma_start(xts[i][:], x_v[bi, i])
                        nc.scalar.copy(xt16s[i][:], xts[i][:])
                        nc.vector.tensor_copy(xrs[i][:], xts[i][:])
                    for t in range(8):
                        for i in range(2):
                            nc.sync.dma_start_transpose(
                                xf16[:, t*F + i*128: t*F + (i+1)*128],
                                xt16s[i][:, t*128:(t+1)*128])

                    for t in range(8):
                        ci = bi * 8 + t
                        S = psum1.tile([128, K], f32, tag="S", name=f"S_{ci}")
                        for h in range(2):
                            hs = slice(h*512, (h+1)*512)
                            for i in range(2):
                                nc.tensor.matmul(S[:, hs],
                                                 xrs[i][:, t*128:(t+1)*128],
                                                 cents2r[i][:, hs],
                                                 start=(i == 0), stop=(i == 1))
                        # T = S - ||c||^2 on DVE; frees S for the next chunk
                        T = work.tile([128, K], f32, tag="T", bufs=2,
                                      name=f"T{ci}")
                        nc.vector.tensor_tensor(out=T[:], in0=S[:],
                                                in1=negc2bc[:],
                                                op=mybir.AluOpType.add)

                        m8 = small.tile([128, 8], f32, tag="m8")
                        nc.vector.max(out=m8[:], in_=T[:])
                        bias = small.tile([128, 1], f32, tag="bias")
                        nc.vector.tensor_scalar(out=bias[:], in0=m8[:, 0:1],
                                                scalar1=-BIG, scalar2=1.0,
                                                op0=mybir.AluOpType.mult,
                                                op1=mybir.AluOpType.add)
                        onehot = work.tile([128, K], f16, tag="onehot", bufs=3)
                        nc.scalar.activation(onehot[:], T[:],
                                             mybir.ActivationFunctionType.Relu,
                                             bias=bias[:], scale=BIG)
                        i8 = small.tile([128, 8], u16, tag="i8")
                        nc.vector.max_index(out=i8[:], in_max=m8[:], in_values=T[:])
                        nc.vector.tensor_copy(ind_all[:, ci:ci+1], i8[:, 0:1])

                        for i in range(2):
                            for h in range(2):
                                nc.tensor.matmul(
                                    segps[i][:, h*512:(h+1)*512],
                                    xf16[:, t*F + i*128: t*F + (i+1)*128],
                                    onehot[:, h*512:(h+1)*512],
                                    start=(ci == 0), stop=(ci == N_CHUNK - 1),
                                    skip_group_check=True)
                        for h in range(2):
                            nc.tensor.matmul(cntps[:, h*512:(h+1)*512],
                                             ones_col16[:],
                                             onehot[:, h*512:(h+1)*512],
                                             start=(ci == 0),
                                             stop=(ci == N_CHUNK - 1),
                                             skip_group_check=True)

                # ------- flush raw partials as fp16 (scale after AR) -------
                for i in range(2):
                    fl = work.tile([128, K], f16, name=f"fl{i}", tag="flush",
                                   bufs=2)
                    nc.vector.tensor_copy(fl[:], segps[i][:])
                    nc.sync.dma_start(ccin[i*128:(i+1)*128, :], fl[:])
                cfl = work.tile([1, K], f16, tag="cflush")
                nc.vector.tensor_copy(cfl[:], cntps[:])
                nc.sync.dma_start(ccin[256:257, :], cfl[:])

            # ---- wrapped gather indices (independent of the collective) ----
            # transpose=False gather: global idx t -> idxs[t%16, t//16].
            # token t = ci*128+p  ->  wrapped16[p%16, ci*8 + p//16]
            w_v = wrapped16.rearrange("q (c u) -> q c u", u=8)
            for u in range(8):
                nc.sync.dma_start(w_v[0:16, :, u],
                                  ind_all[16*u:16*(u+1), :].bitcast(i16))
            for g in range(1, 8):
                nc.sync.dma_start(wrapped16[g*16:(g+1)*16, :], wrapped16[0:16, :])

            # pre-scale avg by DECAY (overlaps the collective)
            avgd = [work.tile([128, K], f32, name=f"avgd{i}", tag=f"avgd{i}")
                    for i in range(2)]
            for i in range(2):
                nc.vector.tensor_scalar_mul(avgd[i][:], avgs[i][:], DECAY)

            # psum1 released; allreduce overlaps the wrapped-idx build
            if KSTAGE >= 2:
                nc.gpsimd.collective_compute(
                    "AllReduce", mybir.AluOpType.add,
                    replica_groups=[list(range(N_CORES))],
                    ins=[ccin.opt()], outs=[ccout.opt()],
                )
            else:
                nc.sync.dma_start(ccout[:], ccin[:])

            with tc.tile_pool(name="psum2", bufs=1, space="PSUM") as psum2:
                # ---- EMA + normalize ----
                seg_g = [work.tile([128, K], f16, name=f"segg{i}", tag=f"segg{i}")
                         for i in range(2)]
                for i in range(2):
                    nc.sync.dma_start(seg_g[i][:], ccout[i*128:(i+1)*128, :])
                cnt8_16 = small.tile([8, 128], f16, tag="cnt8_16")
                nc.sync.dma_start(cnt8_16[:], ccout[256:257, :].rearrange(
                    "one (s p) -> (one s) p", p=128))
                cnt8 = small.tile([8, 128], f32, tag="cnt8")
                nc.vector.tensor_copy(cnt8[:], cnt8_16[:])
                cntT_ps = psum2.tile([128, 8], f32, tag="tp8", name="cntT_ps")
                nc.tensor.transpose(cntT_ps[:], cnt8[:], ident[0:8, 0:8])
                cntT = small.tile([128, 8], f32, tag="cntT")
                # counts were raw; apply (1-DECAY) here
                nc.vector.tensor_scalar_mul(cntT[:], cntT_ps[:], 1.0 - DECAY)
                cs8T_ps = psum2.tile([128, 8], f32, tag="tp8", name="cs8T_ps")
                nc.tensor.transpose(cs8T_ps[:], cs8[:], ident[0:8, 0:8])

                new_csT = small.tile([128, 8], f32, tag="new_csT")
                nc.vector.tensor_scalar_mul(new_csT[:], cs8T_ps[:], DECAY)
                nc.vector.tensor_add(new_csT[:], new_csT[:], cntT[:])
                psum_n = small.tile([128, 1], f32, tag="psum_n")
                nc.vector.reduce_sum(psum_n[:], new_csT[:],
                                     axis=mybir.AxisListType.X)
                n_all = small.tile([128, 1], f32, tag="n_all")
                if KSTAGE >= 3:
                    nc.gpsimd.partition_all_reduce(n_all[:], psum_n[:], channels=128,
                                                   reduce_op=bass_isa.ReduceOp.add)
                else:
                    nc.vector.memset(n_all[:], 550.0)
                # M[k] = 1/cs_norm[k] = (n + K*eps)/n * 1/(new_cs + eps)
                denom = small.tile([128, 1], f32, tag="denom")
                nc.vector.tensor_scalar_add(denom[:], n_all[:], float(K) * EPS)
                rcp_n = small.tile([128, 1], f32, tag="rcp_n")
                nc.vector.reciprocal(rcp_n[:], n_all[:])
                fmul = small.tile([128, 1], f32, tag="fmul")
                nc.vector.tensor_mul(fmul[:], denom[:], rcp_n[:])
                t1 = small.tile([128, 8], f32, tag="t1")
                nc.vector.tensor_scalar_add(t1[:], new_csT[:], EPS)
                r1 = small.tile([128, 8], f32, tag="r1")
                nc.vector.reciprocal(r1[:], t1[:])
                Mt = small.tile([128, 8], f32, tag="Mt")
                nc.vector.tensor_scalar_mul(Mt[:], r1[:], fmul[:])

                newavg = [work.tile([128, K], f32, name=f"newavg{i}",
                                    tag=f"nav{i}") for i in range(2)]
                for i in range(2):
                    # seg_g is raw fp16; newavg = avg*DECAY + seg*(1-DECAY)
                    nc.vector.tensor_scalar_mul(newavg[i][:], seg_g[i][:],
                                                1.0 - DECAY)
                    nc.vector.tensor_add(newavg[i][:], newavg[i][:], avgd[i][:])

                # ---- table: new_centroids^T [K, F] fp16 in DRAM ----
                tabv = tab16_dram.rearrange("(s p) f -> p s f", p=128)
                for s in range(8):
                    tab_sb = work.tile([128, F], f16, tag="tab_sb", bufs=2)
                    for hh in range(2):
                        tps = psum2.tile([128, 128], f32, tag="tps",
                                         name=f"tps{s}_{hh}")
                        nc.tensor.transpose(tps[:],
                                            newavg[hh][:, s*128:(s+1)*128],
                                            ident[:])
                        nc.vector.tensor_scalar_mul(tab_sb[:, hh*128:(hh+1)*128],
                                                    tps[:], Mt[:, s:s+1])
                    nc.sync.dma_start(tabv[:, s, :], tab_sb[:])

                # ---- phase 2: gather (1 desc/idx), PE transpose, write ----
                gath = work.tile([128, N_CHUNK, F], f16, tag="gath")
                n_calls = N_TOK // GQ
                for q in range(n_calls):
                    nc.gpsimd.dma_gather(
                        gath[:, q*(GQ//128):(q+1)*(GQ//128), :],
                        tab16_dram,
                        wrapped16[:, q*(GQ//16):(q+1)*(GQ//16)],
                        num_idxs=GQ, num_idxs_reg=GQ,
                        elem_size=F, transpose=False)

                for bi in range(B_LOC):
                    for i in range(2):
                        out_sb = work.tile([128, HW], f32, name=f"osb{i}",
                                           tag=f"osb{i}", bufs=2)
                        for t in range(8):
                            ci = bi * 8 + t
                            tp = psum2.tile([128, 128], f16, tag=f"tp{i}",
                                            bufs=2, name=f"tp{ci}_{i}")
                            nc.tensor.transpose(
                                tp[:], gath[:, ci, i*128:(i+1)*128], ident16[:])
                            if i == 0:
                                nc.vector.tensor_copy(
                                    out_sb[:, t*128:(t+1)*128], tp[:])
                            else:
                                nc.scalar.copy(
                                    out_sb[:, t*128:(t+1)*128], tp[:])
                        nc.sync.dma_start(out_v[bi, i], out_sb[:])

    nc.finalize()
    return nc


def _get_nc():
    global _NC
    if _NC is None:
        _NC = _build()
    return _NC


def kernel(x, centroids, cluster_size, centroids_avg):
    x = np.ascontiguousarray(np.asarray(x, dtype=np.float32))
    centroids = np.ascontiguousarray(np.asarray(centroids, dtype=np.float32))
    cluster_size = np.ascontiguousarray(np.asarray(cluster_size, dtype=np.float32))
    centroids_avg = np.ascontiguousarray(np.asarray(centroids_avg, dtype=np.float32))
    nc = _get_nc()
    in_maps = []
    for i in range(N_CORES):
        in_maps.append({
            "x": x[i*B_LOC:(i+1)*B_LOC],
            "centroids": centroids,
            "cluster_size": cluster_size,
            "centroids_avg": centroids_avg,
        })
    res = run_bass_kernel_spmd(nc, in_maps, core_ids=list(range(N_CORES)))
    out = np.concatenate([res.results[i]["out"] for i in range(N_CORES)], axis=0)
    return out


if __name__ == "__main__":
    rng = np.random.default_rng(0)
    xs = rng.standard_normal((B, C, H, W), dtype=np.float32)
    cs = rng.standard_normal((C, K), dtype=np.float32)
    sz = rng.random(K, dtype=np.float32)
    av = rng.standard_normal((C, K), dtype=np.float32)
    out = kernel(xs, cs, sz, av)
    print("out", out.shape, out.dtype)
